# revision 1
# baseline (speedup 1.0000x reference)
"""Trainium2 Bass kernel for an 8-head self-attention block with relative
position embeddings (LayerNorm -> qkv -> rel-pos attention -> out proj).

Sharding: pure data-parallel over the batch dim. B == 8 == n_cores, so each
NeuronCore processes one batch element end-to-end; no collectives.

Math notes (per batch element, per head h):
  scores = ((q+u)@k^T + (q+v)@pos^T) / 8
         = (q @ (k+pos)^T + (u.k[m] + v.pos[m])) / 8
so we compute S^T[m,l] = kp_h[m,:] . q_h[l,:] with a per-partition (m) bias
column, which the ACT engine folds into the softmax exp for free:
  P^T = Exp(S_psum * 0.125 + biascol).
Softmax runs in the transposed layout: the denominator Z[l] is obtained by
appending a ones-column to V in the P^T @ V matmul (M=65), and the division
is a K=1 ones-matmul broadcast of 1/Z across partitions followed by a DVE
multiply.  LayerNorm's gamma/beta are folded into the qkv weights on the
host; rstd uses exp(-0.5*ln(var+eps)) so the only ACT tables needed are
ln/exp.

Precision: x/z/w_qkv/q/k and the score matmuls run as float32r (~1e-4 rel
err, full PE rate); the attention-probability side (pe, w_pos inputs, P^T,
v, attn out, w_out) runs bf16 — those errors average out in the softmax-
weighted sums and stay ~1e-3 end to end.

The emission order interleaves the per-dtile projections (q/k/pos/bias/kp)
with the attention of the two heads living in that dtile, so PE matmuls of
tile t+1 overlap the ACT softmax exp of heads 2t, 2t+1.
"""

import math

import numpy as np

MODEL_DIM = 512
NUM_HEADS = 8
DIM_HEAD = 64
INNER = NUM_HEADS * DIM_HEAD
B, L = 8, 1024
EPS = 1e-5
N_CORES = 8
LT = L // 128          # l tiles
KT = MODEL_DIM // 128  # contraction (d) tiles
SCALE = DIM_HEAD ** -0.5

_CACHE = {}


def _rel_pos_emb_np():
    # mirror reference._rel_pos_emb in float32 numpy
    rel = (np.arange(L, dtype=np.float32)[:, None]
           - np.arange(MODEL_DIM, dtype=np.float32)[None, :])
    freqs = np.exp(-np.arange(0, 2 * MODEL_DIM, 2, dtype=np.float32)
                   * np.float32(math.log(10000.0) / MODEL_DIM))
    angle = rel * freqs[None, :]
    even = (np.arange(MODEL_DIM) % 2) == 0
    return np.where(even[None, :], np.cos(angle), np.sin(angle)).astype(np.float32)


def _build():
    import concourse.bacc as bacc
    import concourse.tile as tile
    from concourse import mybir

    F32 = mybir.dt.float32
    F32R = mybir.dt.float32r
    BF16 = mybir.dt.bfloat16
    AF = mybir.ActivationFunctionType
    ALU = mybir.AluOpType

    nc = bacc.Bacc('TRN2', target_bir_lowering=False)

    xb_d = nc.dram_tensor('xb', [L, MODEL_DIM], F32, kind='ExternalInput')
    w1qk_d = nc.dram_tensor('w1qk', [MODEL_DIM, 2 * INNER], F32, kind='ExternalInput')
    w1v_d = nc.dram_tensor('w1v', [MODEL_DIM, INNER], F32, kind='ExternalInput')
    wposT_d = nc.dram_tensor('wposT', [MODEL_DIM, INNER], BF16, kind='ExternalInput')
    peT_d = nc.dram_tensor('peT', [MODEL_DIM, L], BF16, kind='ExternalInput')
    wout2_d = nc.dram_tensor('wout2', [65, NUM_HEADS, MODEL_DIM], BF16, kind='ExternalInput')
    uk_d = nc.dram_tensor('uk', [MODEL_DIM, 2], F32, kind='ExternalInput')
    vp_d = nc.dram_tensor('vp', [MODEL_DIM, 2], F32, kind='ExternalInput')
    ident_d = nc.dram_tensor('ident', [128, 128], F32, kind='ExternalInput')
    ones_d = nc.dram_tensor('ones', [128, 64], F32, kind='ExternalInput')
    onesb_d = nc.dram_tensor('onesb', [128, 64], BF16, kind='ExternalInput')
    onesrow_d = nc.dram_tensor('onesrow', [1, NUM_HEADS * L], BF16, kind='ExternalInput')
    ob_d = nc.dram_tensor('ob', [L, MODEL_DIM], F32, kind='ExternalOutput')

    with nc.allow_low_precision(reason="f32r/bf16 matmul pipeline"), \
            tile.TileContext(nc) as tc:
        with (
            tc.tile_pool(name='const', bufs=1) as constp,
            tc.tile_pool(name='acts', bufs=1) as acts,
            tc.tile_pool(name='wts', bufs=1) as wts,
            tc.tile_pool(name='xz', bufs=4) as xzp,
            tc.tile_pool(name='qkp', bufs=2) as qkp,
            tc.tile_pool(name='posp', bufs=4) as posp,
            tc.tile_pool(name='pt', bufs=2) as ptp,
            tc.tile_pool(name='rc', bufs=4) as rcp,
            tc.tile_pool(name='drc', bufs=4, space='DRAM') as drcp,
            tc.tile_pool(name='outp', bufs=3) as outp,
            tc.tile_pool(name='psM', bufs=2, space='PSUM') as psM,
        ):
            ident_sb = constp.tile([128, 128], F32)
            nc.gpsimd.dma_start(ident_sb[:], ident_d[:])
            ones_sb = constp.tile([128, 64], F32R)
            nc.gpsimd.dma_start(ones_sb[:], ones_d[:].bitcast(F32R))
            eps_sb = constp.tile([128, 1], F32)
            nc.vector.memset(eps_sb[:], EPS)
            # uk/vp tile t holds the 2 selector columns for heads 2t, 2t+1
            uk_sb = constp.tile([128, KT, 2], F32R)
            nc.gpsimd.dma_start(uk_sb[:], uk_d[:].rearrange('(t p) h -> p t h', p=128).bitcast(F32R))
            vp_sb = constp.tile([128, KT, 2], F32R)
            nc.gpsimd.dma_start(vp_sb[:], vp_d[:].rearrange('(t p) h -> p t h', p=128).bitcast(F32R))

            zT = acts.tile([128, KT, L], F32R)
            v_sb = acts.tile([128, LT, NUM_HEADS * 65], BF16)
            biascol = acts.tile([128, LT, NUM_HEADS], F32)
            outT = acts.tile([65, NUM_HEADS, L], BF16)
            wout2_sb = acts.tile([65, NUM_HEADS, MODEL_DIM], BF16)

            w1qk_sb = wts.tile([128, KT, 2 * INNER], F32R)
            w1v_sb = wts.tile([128, KT, INNER], F32R)
            wposT_sb = wts.tile([128, KT, INNER], BF16)
            peT_sb = wts.tile([128, KT, L], BF16)
            w1v_r = w1v_d[:].rearrange('(t p) r -> p t r', p=128).bitcast(F32R)
            w1qk_r = w1qk_d[:].rearrange('(t p) r -> p t r', p=128).bitcast(F32R)
            nc.sync.dma_start(wposT_sb[:], wposT_d[:].rearrange('(t p) r -> p t r', p=128))
            peT_r = peT_d[:].rearrange('(t p) l -> p t l', p=128)
            for kt in range(KT):
                nc.sync.dma_start(peT_sb[:, kt, :], peT_r[:, kt, :])
            for kt in range(KT):
                nc.sync.dma_start(w1v_sb[:, kt, :], w1v_r[:, kt, :])
            for kt in range(KT):
                nc.sync.dma_start(w1qk_sb[:, kt, :], w1qk_r[:, kt, :])
            nc.sync.dma_start(wout2_sb[:], wout2_d[:])
            # ones row = the fused bias/Z row of the out projection
            nc.gpsimd.dma_start(
                outT[64:65, :, :],
                onesrow_d[:].rearrange('o (h l) -> o h l', h=NUM_HEADS))

            # ---------- posT projection (input-independent, fills startup) ----
            posts = {}

            def emit_pos(t, on_act):
                post = posp.tile([128, L], F32R, tag='posT')
                pacc = psM.tile([128, L], F32, tag='S')
                for lc in range(2):
                    for kt in range(KT):
                        nc.tensor.matmul(pacc[:, lc * 512:(lc + 1) * 512],
                                         wposT_sb[:, kt, t * 128:(t + 1) * 128],
                                         peT_sb[:, kt, lc * 512:(lc + 1) * 512],
                                         start=(kt == 0), stop=(kt == KT - 1))
                if on_act:
                    nc.scalar.activation(post[:], pacc[:], AF.Identity)
                else:
                    nc.vector.tensor_copy(post[:], pacc[:])
                posts[t] = post

            emit_pos(0, True)
            emit_pos(1, True)

            # ---------- LayerNorm + transpose into zT ----------
            for lt in range(LT):
                xt = xzp.tile([128, MODEL_DIM], F32, tag='xt')
                nc.scalar.dma_start(xt[:], xb_d[lt * 128:(lt + 1) * 128, :])
                stats = xzp.tile([128, 6], F32, tag='stats')
                nc.vector.bn_stats(stats[:], xt[:])
                mv = xzp.tile([128, 2], F32, tag='mv')
                nc.vector.bn_aggr(mv[:], stats[:])
                lnv = xzp.tile([128, 1], F32, tag='lnv')
                nc.scalar.activation(lnv[:], mv[:, 1:2], AF.Ln, bias=eps_sb[:], scale=1.0)
                rstd = xzp.tile([128, 1], F32, tag='rstd')
                nc.scalar.activation(rstd[:], lnv[:], AF.Exp, scale=-0.5)
                nmr = xzp.tile([128, 1], F32, tag='nmr')
                nc.vector.scalar_tensor_tensor(nmr[:], mv[:, 0:1], -1.0, rstd[:],
                                               op0=ALU.mult, op1=ALU.mult)
                zt = xzp.tile([128, MODEL_DIM], F32, tag='zt')
                nc.scalar.activation(zt[:], xt[:], AF.Identity, bias=nmr[:], scale=rstd[:])
                tp = psM.tile([128, 512], F32, tag='qkv')
                for c in range(KT):
                    nc.tensor.transpose(tp[:, c * 128:(c + 1) * 128],
                                        zt[:, c * 128:(c + 1) * 128], ident_sb[:])
                nc.vector.tensor_copy(
                    zT[:, :, lt * 128:(lt + 1) * 128],
                    tp[:].rearrange('p (c l) -> p c l', c=KT))

            emit_pos(2, False)
            emit_pos(3, False)
            # ones column per head block
            nc.gpsimd.dma_start(
                v_sb[:].rearrange('p t (h c) -> p t h c', c=65)[:, :, :, 64:65],
                onesb_d[:].rearrange('p (t h o) -> p t h o', t=LT, h=NUM_HEADS))

            # ---------- interleaved projections + attention ----------
            for t in range(KT):
                qt = qkp.tile([128, L], F32R, tag='qT')
                ktt = qkp.tile([128, L], F32R, tag='kT')
                post = posts[t]
                # q, k projections for dtile t
                for which, dst in ((0, qt), (1, ktt)):
                    rt = which * 4 + t
                    for lc in range(2):
                        acc = psM.tile([128, 512], F32, tag='qkv')
                        for kt in range(KT):
                            nc.tensor.matmul(acc[:], w1qk_sb[:, kt, rt * 128:(rt + 1) * 128],
                                             zT[:, kt, lc * 512:(lc + 1) * 512],
                                             start=(kt == 0), stop=(kt == KT - 1))
                        nc.vector.tensor_copy(dst[:, lc * 512:(lc + 1) * 512], acc[:])
                # bias columns for heads 2t, 2t+1
                for mc in range(LT):
                    acc = psM.tile([128, 2], F32, tag='qkv')
                    nc.tensor.matmul(acc[:], ktt[:, mc * 128:(mc + 1) * 128],
                                     uk_sb[:, t, :], start=True, stop=False)
                    nc.tensor.matmul(acc[:], post[:, mc * 128:(mc + 1) * 128],
                                     vp_sb[:, t, :], start=False, stop=True)
                    nc.vector.tensor_copy(biascol[:, mc, 2 * t:2 * t + 2], acc[:])
                # kp = k + pos for dtile t (in place on ktt)
                nc.vector.tensor_tensor(ktt[:], ktt[:], post[:], op=ALU.add)

                if t == 0:
                    # v projection (needed first by PV of head 0)
                    for mt in range(LT):
                        acc = psM.tile([128, 512], F32, tag='qkv')
                        for kt in range(KT):
                            nc.tensor.matmul(acc[:], zT[:, kt, mt * 128:(mt + 1) * 128],
                                             w1v_sb[:, kt, :],
                                             start=(kt == 0), stop=(kt == KT - 1))
                        nc.vector.tensor_copy(
                            v_sb[:, mt, :].rearrange('p (h c) -> p h c', c=65)[:, :, 0:64],
                            acc[:].rearrange('p (h c) -> p h c', c=64))

                # attention for the two heads of dtile t
                for h in (2 * t, 2 * t + 1):
                    hp = 64 * (h % 2)
                    prow = slice(hp, hp + 64)
                    PT = ptp.tile([128, LT, L], BF16, tag='PT')
                    for mt in range(LT):
                        sacc = psM.tile([128, L], F32, tag='S')
                        for lc in range(2):
                            nc.tensor.matmul(sacc[:, lc * 512:(lc + 1) * 512],
                                             ktt[prow, mt * 128:(mt + 1) * 128],
                                             qt[prow, lc * 512:(lc + 1) * 512],
                                             start=True, stop=True)
                        nc.scalar.activation(PT[:, mt, :], sacc[:], AF.Exp,
                                             bias=biascol[:, mt, h:h + 1], scale=SCALE)
                    for lc in range(2):
                        ls = slice(lc * 512, (lc + 1) * 512)
                        pvacc = psM.tile([65, 512], F32, tag='pv')
                        for mt in range(LT):
                            nc.tensor.matmul(pvacc[:], v_sb[:, mt, h * 65:(h + 1) * 65],
                                             PT[:, mt, ls],
                                             start=(mt == 0), stop=(mt == LT - 1))
                        rc = rcp.tile([128, 512], F32, tag='rc')
                        if h < 6:
                            nc.vector.reciprocal(rc[64:65, :], pvacc[64:65, :])
                            rcd = drcp.tile([1, 512], F32, tag='rcd')
                            nc.sync.dma_start(rcd[:], rc[64:65, :])
                            rcb = rcp.tile([64, 512], F32, tag='rcb')
                            nc.sync.dma_start(rcb[:], rcd[:].to_broadcast((64, 512)))
                            nc.vector.tensor_mul(outT[0:64, h, ls], pvacc[0:64, :], rcb[:])
                        else:
                            # low-latency path for the last heads: K=1 ones-matmul
                            # broadcast keeps the out-projection off the DMA chain
                            rcr = rc.bitcast(F32R)
                            nc.vector.reciprocal(rcr[64:65, :], pvacc[64:65, :])
                            bcacc = psM.tile([64, 512], F32, tag='pv')
                            nc.tensor.matmul(bcacc[:], ones_sb[64:65, :], rcr[64:65, :],
                                             start=True, stop=True)
                            pvs = rcp.tile([64, 512], F32, tag='rcb')
                            nc.vector.tensor_copy(pvs[:], pvacc[0:64, :])
                            nc.vector.tensor_mul(outT[0:64, h, ls], bcacc[:], pvs[:])

            # ---------- output projection ----------
            for lt in range(LT):
                facc = psM.tile([128, MODEL_DIM], F32, tag='qkv')
                for h in range(NUM_HEADS):
                    nc.tensor.matmul(facc[:], outT[:, h, lt * 128:(lt + 1) * 128],
                                     wout2_sb[:, h, :],
                                     start=(h == 0), stop=(h == NUM_HEADS - 1))
                ot = outp.tile([128, MODEL_DIM], F32, tag='ot')
                nc.scalar.activation(ot[:], facc[:], AF.Identity)
                nc.scalar.dma_start(ob_d[lt * 128:(lt + 1) * 128, :], ot[:])

    # Force all activations (Ln/Exp/Identity) onto the single table set that
    # contains them all — otherwise the table-load picker alternates between
    # the natural_log and exp sets, paying a ~2.7us table load per switch.
    import concourse.bacc as bacc_mod
    orig_tables = bacc_mod.get_activation_tables

    def _only_ln_exp(arch):
        t = orig_tables(arch)
        return {name: (funcs if name == 'natural_log_exp_and_others' else
                       type(funcs)())
                for name, funcs in t.items()}

    bacc_mod.get_activation_tables = _only_ln_exp
    try:
        nc.compile()
    finally:
        bacc_mod.get_activation_tables = orig_tables
    return nc


def _host_prep(x, gamma, beta, w_qkv, b_qkv, w_pos, w_out, b_out, u_bias, v_bias):
    """Host-side layout prep. Returns (common_inputs, per_core_x_list)."""
    import ml_dtypes
    BF = ml_dtypes.bfloat16
    W1 = (gamma[:, None] * w_qkv.T).astype(np.float32)        # [D, 3*INNER]
    b1 = (b_qkv + beta @ w_qkv.T).astype(np.float32)
    if np.any(b1 != 0):
        raise NotImplementedError("nonzero qkv bias not supported by this kernel")
    w1qk = np.ascontiguousarray(W1[:, :2 * INNER])
    w1v = np.ascontiguousarray(W1[:, 2 * INNER:])
    wposT = np.ascontiguousarray(w_pos.T).astype(BF)
    peT = np.ascontiguousarray(_rel_pos_emb_np().T).astype(BF)
    wout2 = np.zeros((65, NUM_HEADS, MODEL_DIM), np.float32)
    for h in range(NUM_HEADS):
        wout2[0:64, h, :] = w_out.T[h * 64:(h + 1) * 64, :]
    wout2[64, 0, :] = b_out
    wout2 = wout2.astype(BF)
    # uk/vp: [D, 2]; rows 128t+64c .. +64 hold u/v of head 2t+c in column c
    uk = np.zeros((MODEL_DIM, 2), np.float32)
    vp = np.zeros((MODEL_DIM, 2), np.float32)
    for h in range(NUM_HEADS):
        t, half = divmod(h, 2)
        uk[t * 128 + half * 64: t * 128 + half * 64 + 64, half] = u_bias[h] * SCALE
        vp[t * 128 + half * 64: t * 128 + half * 64 + 64, half] = v_bias[h] * SCALE
    common = {
        'w1qk': w1qk, 'w1v': w1v, 'wposT': wposT, 'peT': peT, 'wout2': wout2,
        'uk': uk, 'vp': vp,
        'ident': np.eye(128, dtype=np.float32),
        'ones': np.ones((128, 64), np.float32),
        'onesb': np.ones((128, 64), BF),
        'onesrow': np.ones((1, NUM_HEADS * L), BF),
    }
    xs = [np.ascontiguousarray(x[b]) for b in range(N_CORES)]
    return common, xs


def kernel(x, gamma, beta, w_qkv, b_qkv, w_pos, w_out, b_out, u_bias, v_bias):
    x = np.asarray(x, np.float32)
    args = [np.asarray(a, np.float32) for a in
            (gamma, beta, w_qkv, b_qkv, w_pos, w_out, b_out, u_bias, v_bias)]
    common, xs = _host_prep(x, *args)

    if 'nc' not in _CACHE:
        _CACHE['nc'] = _build()
    nc = _CACHE['nc']

    from concourse.bass_utils import run_bass_kernel_spmd
    in_maps = [{'xb': xs[b], **common} for b in range(N_CORES)]
    res = run_bass_kernel_spmd(nc, in_maps, core_ids=list(range(N_CORES)))
    return np.stack([res.results[b]['ob'] for b in range(N_CORES)], axis=0)

